# revision 2
# baseline (speedup 1.0000x reference)
"""DGALoss Trainium kernel v4 — PE computes the residuals in PSUM.

Math (validated vs the jax reference, rel ~3e-4; tolerance 2e-2): angles
are tiny, so BCH to FIRST order is enough:
    t4 = rs4/HUBER = 200*xs[::16] - 2*sum16(w_hat)     (DT/HUBER = 2)
    t5 = rs5/HUBER = 200*xs5      - 2*sum32(w_hat),  xs5 = pairsum(xs4)
Everything linear runs on the otherwise-idle PE as accumulating matmuls
into PSUM f32:
  - w_hat fp8(e4m3), partition-transposed (16 consecutive samples in 16
    consecutive partitions), as 96-col ldweights chunks; block-(-2)
    moving operands [128,8]/[128,4] (packed into wh cols 0:12) produce
    -2*sum16 / -2*sum32.
  - a 200*I[96,96] stationary (packed into xs cols 512:608) with xs
    column slices as moving data adds the 200*xs terms.
So each chunk's PSUM tile holds t4/t5 with no vector-engine work.

SmoothL1: h = a + 0.5*(m-2)*m, a=|t|, m=min(a,1), per chunk:
  ACT: Abs+accum on psum -> Sa;  DVE: m = (t abs_max 0) min 1 in one
  tensor_scalar;  Pool: STT (m-2)*m + accum -> Sw.
The [:, N0:] row mask zeroes masked PSUM cells (h(0)=0) via tiny
memsets (ps4 on DVE, ps5 on Pool); the last chunk has none.

Chunks use separate PSUM tiles (8 tiles = 8 banks) so their matmul
groups and Huber chains are fully independent.  Wire order: wh c0
(HWDGE) -> xs (HWDGE) -> wh c1 (Pool SWDGE, prepped early) -> c2 -> c3;
the last chunk is 6 blocks so only a short chain trails the final DMA
semaphore.

Per-core I/O: wh [128,6156] fp8, xs [96,608] bf16,
out [96,16] f32 = (Sa4,Sw4,Sa5,Sw5) x 4 chunks.
"""

import numpy as np
import ml_dtypes

# ---- problem constants (hardcoded per spec) ----
N_ROWS = 64
T = 32768
N_CORES = 8
ROWS_PER_CORE = N_ROWS // N_CORES          # 8
DT = 0.01
HUBER = 0.005
W_CONST = 1.0e6
N0 = 5
N4 = N_ROWS * (T // 16 - N0) * 3           # 392256 valid level-4 elements
N5 = N_ROWS * (T // 32 - N0) * 3           # 195648 valid level-5 elements

MM_CH = [25, 25, 8, 6]                     # chunk sizes in 96-col blocks
# masked rows (N0 leading L4/L5 groups of each of the 8 batch rows) per
# chunk for this split: chunk -> (r_lo, r_hi)
MASK_R = {0: (0, 4), 1: (4, 7), 2: (7, 8)}

_CACHE = {}


def _build():
    import concourse.bass as bass
    import concourse.tile as tile
    from concourse import mybir

    f32 = mybir.dt.float32
    bf16 = mybir.dt.bfloat16
    fp8 = mybir.dt.float8e4
    AF = mybir.ActivationFunctionType
    OP = mybir.AluOpType
    AX = mybir.AxisListType

    nc = bass.Bass()
    wh_d = nc.dram_tensor("wh", [128, 6156], fp8, kind="ExternalInput")
    xs_d = nc.dram_tensor("xs", [96, 608], bf16, kind="ExternalInput")
    out_d = nc.dram_tensor("out", [96, 16], f32, kind="ExternalOutput")

    k0s = np.cumsum([0] + MM_CH).tolist()

    with tile.TileContext(nc) as tc:
        with tc.tile_pool(name="main", bufs=1) as pool, \
             tc.tile_pool(name="psum", bufs=1, space="PSUM") as psp:
            V = nc.vector
            S = nc.scalar
            Gp = nc.gpsimd

            def tl(shape, tag, dtp=bf16):
                return pool.tile(shape, dtp, name=tag, tag=tag)

            wh_t = tl([128, 6156], "wh_t", fp8)
            xs_t = tl([96, 608], "xs_t")
            m4 = tl([96, 512], "m4")
            m5 = tl([96, 256], "m5")
            a4 = tl([96, 512], "a4")
            a5 = tl([96, 256], "a5")
            s4 = tl([96, 512], "s4")
            s5 = tl([96, 256], "s5")
            out_t = tl([96, 16], "out_t", f32)
            # chunk-local PSUM tiles (8 tiles = 8 banks); padded to 64/32-col
            # multiples so the strided mask views below stay in bounds (the
            # pad cells are never read or written)
            ps4c = [psp.tile([96, ((8 * n + 63) // 64) * 64], f32,
                             name=f"ps4_{i}", tag=f"ps4_{i}")
                    for i, n in enumerate(MM_CH)]
            ps5c = [psp.tile([96, ((4 * n + 31) // 32) * 32], f32,
                             name=f"ps5_{i}", tag=f"ps5_{i}")
                    for i, n in enumerate(MM_CH)]

            # constants ride along with the uploads
            n2_16 = wh_t[:, 0:8]
            n2_32 = wh_t[:, 8:12]
            i200 = xs_t[:, 512:608]
            xs_m = xs_t[:, 0:512]
            xs_e = xs_m.rearrange("p (j e) -> p j e", e=2)[:, :, 0]
            xs_o = xs_m.rearrange("p (j e) -> p j e", e=2)[:, :, 1]

            # DMA plan: xs leads on HWDGE; chunk0 rides the Pool SWDGE
            # queue (its prep runs during the preamble) so it follows xs on
            # the wire immediately; chunks 1-3 stream behind on HWDGE with
            # the small chunks last.
            edges = [0] + [12 + 96 * k for k in k0s[1:]]
            nc.sync.dma_start(out=xs_t[:, :], in_=xs_d[:, :])
            Gp.dma_start(out=wh_t[:, edges[0]:edges[1]],
                         in_=wh_d[:, edges[0]:edges[1]])
            nc.sync.dma_start(out=wh_t[:, edges[1]:edges[2]],
                              in_=wh_d[:, edges[1]:edges[2]])
            nc.sync.dma_start(out=wh_t[:, edges[2]:edges[3]],
                              in_=wh_d[:, edges[2]:edges[3]])
            nc.sync.dma_start(out=wh_t[:, edges[3]:edges[4]],
                              in_=wh_d[:, edges[3]:edges[4]])

            def rview(ps, off, stride, nr, width):
                """cols off + stride*j + [0, width), j in [0, nr)."""
                if nr == 1:
                    return ps[:, off:off + width]
                v = ps[:, off:off + stride * nr]
                return v.rearrange("p (r x) -> p r x", r=nr)[:, :, 0:width]

            for ci, nmm in enumerate(MM_CH):
                k0, k1 = k0s[ci], k0s[ci + 1]
                p4, p5 = ps4c[ci], ps5c[ci]
                w4c, w5c = 8 * nmm, 4 * nmm
                # each 8-col (4-col) PSUM region is one accumulation
                # group: -2*group-sum starts it, the 200*xs identity-matmul
                # terms close it.  Group members MUST be emitted
                # back-to-back: separating them lets the scheduler break
                # the start/accumulate ordering (verified empirically).
                c4g = slice(8 * k0, 8 * k1)
                c5g = slice(4 * k0, 4 * k1)
                for k in range(k0, k1):
                    lw = wh_t[:, 12 + 96 * k:12 + 96 * k + 96]
                    j = k - k0
                    nc.tensor.matmul(p4[:, 8 * j:8 * j + 8], lw, n2_16,
                                     start=True, stop=False)
                    nc.tensor.matmul(p4[:, 8 * j:8 * j + 8], i200,
                                     xs_m[:, 8 * k:8 * k + 8],
                                     start=False, stop=True)
                    nc.tensor.matmul(p5[:, 4 * j:4 * j + 4], lw, n2_32,
                                     start=True, stop=False)
                    nc.tensor.matmul(p5[:, 4 * j:4 * j + 4], i200,
                                     xs_e[:, 4 * k:4 * k + 4],
                                     start=False, stop=False)
                    nc.tensor.matmul(p5[:, 4 * j:4 * j + 4], i200,
                                     xs_o[:, 4 * k:4 * k + 4],
                                     start=False, stop=True)
                # N0 row-mask memsets on the chunk-local PSUM cells
                if ci in MASK_R:
                    r_lo, r_hi = MASK_R[ci]
                    nr = r_hi - r_lo
                    off4 = 64 * r_lo - 8 * k0
                    off5 = 32 * r_lo - 4 * k0
                    V.memset(rview(p4, off4, 64, nr, 5)[0:3], 0.0)
                    V.memset(rview(p5, off5, 32, nr, 4)[0:3], 0.0)
                    V.memset(rview(p5, off5, 32, nr, 1)[0:6], 0.0)
                # Huber
                # out cols per chunk: 0=Sa4 (ACT), 1=Sw4 (Pool), 2=Sa5
                # (DVE reduce-abs, keeps the second accumulator-read off
                # ACT), 3=Sw5 (Pool)
                S.activation(a4[:, c4g], p4[:, 0:w4c], AF.Abs,
                             accum_out=out_t[:, 4 * ci:4 * ci + 1])
                V.tensor_scalar(m4[:, c4g], a4[:, c4g], 1.0, None, OP.min)
                S.activation(a5[:, c5g], p5[:, 0:w5c], AF.Abs,
                             accum_out=out_t[:, 4 * ci + 2:4 * ci + 3])
                V.tensor_scalar(m5[:, c5g], a5[:, c5g], 1.0, None, OP.min)
                V.scalar_tensor_tensor(s4[:, c4g], m4[:, c4g], -2.0,
                                       m4[:, c4g], OP.add, OP.mult,
                                       accum_out=out_t[:, 4 * ci + 1:
                                                       4 * ci + 2])
                V.scalar_tensor_tensor(s5[:, c5g], m5[:, c5g], -2.0,
                                       m5[:, c5g], OP.add, OP.mult,
                                       accum_out=out_t[:, 4 * ci + 3:
                                                       4 * ci + 4])

            nc.sync.dma_start(out=out_d[:, :], in_=out_t[:, :])

    _legalize_waits(nc)
    return nc


def _legalize_waits(nc):
    """walrus TPB descriptors hold ONE sync-wait slot per instruction.
    Split excess waits onto same-engine NoOps ahead of the instruction."""
    from concourse import mybir

    for f in nc.m.functions:
        for blk in f.blocks:
            insts = blk.instructions
            idx = 0
            while idx < len(insts):
                inst = insts[idx]
                si = getattr(inst, "sync_info", None)
                if si is None or not si.on_wait:
                    idx += 1
                    continue
                waits = list(si.on_wait)
                if len(waits) <= 1:
                    idx += 1
                    continue
                extra, keep = waits[:-1], waits[-1:]
                for w in extra:
                    nop = mybir.InstNoOp(
                        name=nc.get_next_instruction_name(),
                        ins=[],
                        outs=[],
                        engine=inst.engine,
                        sync_info=mybir.SyncInfo(on_wait=[w], on_update=[]),
                        bass_nofuse=True,
                    )
                    nc.register_instruction(nop)
                    blk.instructions.insert(idx, nop)
                    idx += 1
                si.on_wait = keep
                idx += 1


def _run(in_maps, trace=False, tmpdir=None):
    from concourse.bass_utils import run_bass_kernel_spmd

    if "nc" not in _CACHE:
        _CACHE["nc"] = _build()
    nc = _CACHE["nc"]
    return run_bass_kernel_spmd(nc, in_maps, list(range(N_CORES)),
                                trace=trace, tmpdir=tmpdir)


def _shard(xs, w_hat):
    xs = np.ascontiguousarray(xs, dtype=np.float32)
    w_hat = np.ascontiguousarray(w_hat, dtype=np.float32)
    in_maps = []
    for c in range(N_CORES):
        r0, r1 = c * ROWS_PER_CORE, (c + 1) * ROWS_PER_CORE
        # whT[p, 12+3t+comp] = w_hat item (t*128+p): 16 consecutive samples
        # sit in 16 consecutive partitions for the PE group-sums; cols 0:12
        # carry the block-(-2) moving operands
        whc = np.zeros((128, 6156), dtype=ml_dtypes.float8_e4m3)
        for g in range(8):
            whc[16 * g:16 * (g + 1), g] = -2.0
        for h in range(4):
            whc[32 * h:32 * (h + 1), 8 + h] = -2.0
        whc[:, 12:] = (w_hat[r0:r1].reshape(2048, 128, 3).transpose(1, 0, 2)
                       .reshape(128, 6144).astype(ml_dtypes.float8_e4m3))
        # xsT[3u+comp, 8c+g] = xs4 group (32c+u)*8+g, matching the PE
        # output; cols 512:608 carry 200*I[96]
        xsc = np.zeros((96, 608), dtype=ml_dtypes.bfloat16)
        xsc[:, 0:512] = (xs[r0:r1, ::16].reshape(64, 32, 8, 3)
                         .transpose(1, 3, 0, 2).reshape(96, 512)
                         .astype(ml_dtypes.bfloat16))
        xsc[:, 512:608] = (200.0 * np.eye(96, dtype=np.float32)
                           ).astype(ml_dtypes.bfloat16)
        in_maps.append({"wh": np.ascontiguousarray(whc),
                        "xs": np.ascontiguousarray(xsc)})
    return in_maps


def _combine(results):
    sa4 = sw4 = sa5 = sw5 = 0.0
    for r in results:
        o = np.asarray(r["out"], dtype=np.float64)
        sa4 += o[:, 0::4].sum()
        sw4 += o[:, 1::4].sum()
        sa5 += o[:, 2::4].sum()
        sw5 += o[:, 3::4].sum()
    loss = W_CONST * HUBER * HUBER * ((sa4 + 0.5 * sw4) / N4
                                      + 0.5 * (sa5 + 0.5 * sw5) / N5)
    return np.array(loss, dtype=np.float32)


def kernel(xs, w_hat):
    res = _run(_shard(xs, w_hat))
    return _combine(res.results)
